# revision 19
# baseline (speedup 1.0000x reference)
"""Trainium2 Bass kernel for nn_AttentionHead (B=4, S=2048, E=2048, DH=256).

v3: padded-key compaction + bf16 + host-side normalization + tuned overlap.

Sharding: 8 cores = (batch b, query-parity h). Core (b, h) owns the 1024
queries in 128-row blocks {h, h+2, ..., h+14} and attends over the COMPACTED
key list of batch b (padding-masked keys dropped entirely — exact: they get
zero softmax weight). ~half the keys are padding, so scores/AV/denominator
work halves.

All heavy matmuls run in bf16. Host packs x/weights so every DMA is
8KB-contiguous per partition; input streams are spread over the three DMA
queues (SP: xkv, ACT: xq, Pool/SWDGE: weights+masks) in consumption order.

Scores are computed transposed (S^T[k, q]). Padding-tail key rows are masked
via the per-partition exp bias (free on ACT); only the causal-partial column
band of each tile gets a 2D mask add on DVE. exp/AV are split at the band
edge so the unmasked part never waits on DVE. Phase A is software-pipelined
(score matmuls run 2 tiles ahead of the P*V consumers). The device returns
the UNNORMALIZED numerator O^T = V^T P and denominators; the host divides
and adds b_V.
"""
import sys

sys.path.insert(0, "/opt/trn_rl_repo")

import numpy as np

import concourse.bacc as bacc
import concourse.mybir as mybir
import concourse.tile as tile

F32 = mybir.dt.float32
BF16 = mybir.dt.bfloat16
AF = mybir.ActivationFunctionType
ALU = mybir.AluOpType

B, S, E, DH = 4, 2048, 2048, 256
SQ = S // 2          # queries per core
EC = E // 128        # contraction chunks (16)
NEG = -1.0e30
SCALE = 1.0 / 16.0   # 1/sqrt(DH)

TRACE = False
LAST_RESULTS = None

_prog_cache = {}


def _qpos(h):
    """Global query positions owned by core-parity h (interleaved blocks)."""
    return np.concatenate(
        [np.arange((2 * j + h) * 128, (2 * j + h + 1) * 128) for j in range(8)]
    )


def _build_plan(causal, pad):
    """Static SPMD structure (shared by all cores) + per-core mask data."""
    kpos = [np.where(~pad[b])[0] for b in range(B)]
    nv = [len(k) for k in kpos]
    NKT = (max(nv) + 127) // 128
    VK = NKT * 128

    groups = []
    off = 0
    while off < VK:
        w = min(512, VK - off)
        groups.append((off, w))
        off += w

    # per-core transposed validity [key-slot, query-col]
    vTs = []
    for c in range(8):
        b, h = c // 2, c % 2
        qp = _qpos(h)
        vT = np.zeros((VK, SQ), dtype=bool)
        vT[: nv[b]] = causal[np.ix_(qp, kpos[b])].T
        vTs.append(vT)

    # visit set: union over cores.
    #   qs = first query col with any valid entry (min over cores)
    #   cm = first query col from which every core's tile-cols are fully valid
    #        among live rows (rows < nv; dead rows are handled by the exp
    #        bias) — only [qs, cm) needs the 2D mask add.
    visits = []   # [(qg, [(p, qs, cm, moff), ...]), ...]
    moff = 0
    for qg in range(2):
        tl = []
        for p in range(NKT):
            anyv = np.zeros(512, dtype=bool)
            fullv = np.ones(512, dtype=bool)
            for ci, vT in enumerate(vTs):
                b = ci // 2
                live = min(128, max(0, nv[b] - p * 128))
                seg = vT[p * 128:(p + 1) * 128, qg * 512:(qg + 1) * 512]
                anyv |= seg.any(axis=0)
                fullv &= seg[:live].all(axis=0)
            if not anyv.any():
                continue
            qs = int(np.argmax(anyv))
            # cm: first col such that fullv[cm:] is all True
            idx = np.where(~fullv)[0]
            cm = int(idx[-1]) + 1 if len(idx) else qs
            cm = max(cm, qs)
            tl.append((p, qs, cm, moff))
            moff += cm - qs
        visits.append((qg, tl))
    totm = max(moff, 8)

    key = (VK, tuple(groups),
           tuple((qg, tuple(tl)) for qg, tl in visits))
    return dict(kpos=kpos, nv=nv, NKT=NKT, VK=VK, groups=groups,
                visits=visits, totm=totm, vTs=vTs, key=key)


def _build_program(plan):
    nc = bacc.Bacc("TRN2", target_bir_lowering=False, debug=False, num_devices=8)
    VK, NKT, totm = plan["VK"], plan["NKT"], plan["totm"]

    t = {}
    t["xq"] = nc.dram_tensor("xq", [128, EC * SQ], BF16, kind="ExternalInput").ap()
    t["xkv"] = nc.dram_tensor("xkv", [128, EC * VK], BF16, kind="ExternalInput").ap()
    t["wq"] = nc.dram_tensor("wq", [128, EC * DH], BF16, kind="ExternalInput").ap()
    t["wk"] = nc.dram_tensor("wk", [128, EC * DH], BF16, kind="ExternalInput").ap()
    t["wv"] = nc.dram_tensor("wv", [128, EC * DH], BF16, kind="ExternalInput").ap()
    t["bq"] = nc.dram_tensor("bq", [128, 2], F32, kind="ExternalInput").ap()
    t["bk"] = nc.dram_tensor("bk", [128, 2], F32, kind="ExternalInput").ap()
    t["pbias"] = nc.dram_tensor("pbias", [128, NKT], F32, kind="ExternalInput").ap()
    t["ones"] = nc.dram_tensor("ones", [128, 128], BF16, kind="ExternalInput").ap()
    t["maskc"] = nc.dram_tensor("maskc", [128, totm], BF16, kind="ExternalInput").ap()
    t["out"] = nc.dram_tensor("out", [128, 2 * SQ], BF16, kind="ExternalOutput").ap()
    t["odn"] = nc.dram_tensor("odn", [1, SQ], F32, kind="ExternalOutput").ap()

    with tile.TileContext(nc) as tc:
        _emit(nc, tc, t, plan)
    nc.compile()
    return nc


def _emit(nc, tc, t, plan):
    from contextlib import ExitStack

    VK, NKT = plan["VK"], plan["NKT"]
    groups, visits, totm = plan["groups"], plan["visits"], plan["totm"]

    with ExitStack() as ctx:
        const = ctx.enter_context(tc.tile_pool(name="const", bufs=1))
        persist = ctx.enter_context(tc.tile_pool(name="persist", bufs=1))

        # ---- constants (Pool/SWDGE ring; wk head-chunk first) ----------
        wk_sb = const.tile([128, EC, DH], BF16, tag="wk")
        wq_sb = const.tile([128, EC, DH], BF16, tag="wq")
        wv_sb = const.tile([128, EC, DH], BF16, tag="wv")
        wk_r = t["wk"].rearrange("p (c d) -> p c d", c=EC)
        nc.gpsimd.dma_start(wk_sb[:, 0:2, :], wk_r[:, 0:2, :])
        nc.gpsimd.dma_start(wk_sb[:, 2:8, :], wk_r[:, 2:8, :])
        nc.gpsimd.dma_start(wk_sb[:, 8:, :], wk_r[:, 8:, :])
        nc.gpsimd.dma_start(wv_sb[:], t["wv"].rearrange("p (c d) -> p c d", c=EC))
        nc.gpsimd.dma_start(wq_sb[:], t["wq"].rearrange("p (c d) -> p c d", c=EC))
        bq_sb = const.tile([128, 2], F32, tag="bq")
        bk_sb = const.tile([128, 2], F32, tag="bk")
        pb_sb = const.tile([128, NKT], F32, tag="pb")
        ones_sb = const.tile([128, 128], BF16, tag="ones")
        nc.gpsimd.dma_start(bq_sb[:], t["bq"][:])
        nc.gpsimd.dma_start(bk_sb[:], t["bk"][:])
        nc.gpsimd.dma_start(pb_sb[:], t["pbias"][:])
        nc.gpsimd.dma_start(ones_sb[:], t["ones"][:])
        mask_sb = const.tile([128, totm], BF16, tag="mask")

        # ---- persistent SBUF tensors
        kt_sb = persist.tile([128, 2, VK], BF16, tag="ktp")
        qt_sb = persist.tile([128, 2, SQ], BF16, tag="qtp")
        v_sb = persist.tile([128, NKT, DH], BF16, tag="vp")
        o_sb = persist.tile([128, 2, SQ], BF16, tag="osb")
        dn_sb = persist.tile([128, SQ], F32, tag="dnsb")

        xq_r = t["xq"].rearrange("p (g c s) -> p g c s", g=2, c=EC)

        # ---- phase P: projections -------------------------------------
        # Per super-group: K (2 dh-halves), Q (2 dh-halves), V (4 key
        # tiles).  xkv chunks stream on SP, xq on ACT, weights on Pool.
        with tc.tile_pool(name="xkv", bufs=2) as xkv_pool, \
             tc.tile_pool(name="xq", bufs=2) as xq_pool, \
             tc.tile_pool(name="pp", bufs=2, space="PSUM") as pp, \
             tc.tile_pool(name="vp", bufs=2, space="PSUM") as vp:

            def load_xkv(gi, csplit):
                goff, w = groups[gi]
                xkt = xkv_pool.tile([128, EC, 512], BF16, tag="xkt")
                c0 = 0
                for cw in csplit:
                    foff = goff * EC + c0 * w
                    nc.sync.dma_start(
                        xkt[:, c0:c0 + cw, :w],
                        t["xkv"][:, foff:foff + cw * w].rearrange(
                            "p (c s) -> p c s", c=cw),
                    )
                    c0 += cw
                return xkt

            def load_xq(g):
                xqt = xq_pool.tile([128, EC, 512], BF16, tag="xqt")
                for ch in range(2):
                    nc.scalar.dma_start(
                        xqt[:, ch * 8:(ch + 1) * 8, :],
                        xq_r[:, g, ch * 8:(ch + 1) * 8, :],
                    )
                return xqt

            def proj(w_sb, b_sb, dst, dh2, xt, w):
                ps = pp.tile([128, 512], F32, tag="pp")
                for e in range(EC):
                    nc.tensor.matmul(
                        ps[:, :w], w_sb[:, e, dh2 * 128:(dh2 + 1) * 128],
                        xt[:, e, :w], start=(e == 0), stop=(e == EC - 1),
                    )
                nc.scalar.activation(dst, ps[:, :w], AF.Identity,
                                     bias=b_sb[:, dh2:dh2 + 1])

            def vproj(xt, goff, w):
                for kt4 in range(w // 128):
                    psv = vp.tile([128, DH], F32, tag="vps")
                    for e in range(EC):
                        nc.tensor.matmul(
                            psv[:], xt[:, e, kt4 * 128:(kt4 + 1) * 128],
                            wv_sb[:, e, :], start=(e == 0), stop=(e == EC - 1),
                        )
                    nc.vector.tensor_copy(v_sb[:, goff // 128 + kt4, :], psv[:])

            # prefetch order on each ring == consumption order
            xk0 = load_xkv(0, [2, 2, 4, 4, 4])
            xq0 = load_xq(0)
            xk1 = load_xkv(1, [8, 8]) if len(groups) > 1 else None
            xq1 = load_xq(1)
            xk2 = load_xkv(2, [16]) if len(groups) > 2 else None
            # masks ride the SP ring after all xkv (needed only in phase A)
            nc.sync.dma_start(mask_sb[:], t["maskc"][:])

            for gi, xkt in enumerate([xk0, xk1, xk2][:len(groups)]):
                goff, w = groups[gi]
                for dh2 in range(2):
                    proj(wk_sb, bk_sb, kt_sb[:, dh2, goff:goff + w], dh2, xkt, w)
                if gi < 2:
                    xqt = xq0 if gi == 0 else xq1
                    for dh2 in range(2):
                        proj(wq_sb, bq_sb, qt_sb[:, dh2, gi * 512:(gi + 1) * 512],
                             dh2, xqt, 512)
                vproj(xkt, goff, w)

        # ---- phase A: attention ---------------------------------------
        # Flatten tiles across both query-groups; software-pipeline: score
        # matmuls run LOOKAHEAD tiles ahead of the mask/exp/AV consumers.
        LOOK = 2
        flat = []
        for qg, tl in visits:
            nt = len(tl)
            # staged flush: cols < next tile's qs are final after this tile;
            # flush in >=256-col pieces so the tail only handles the last one
            lo = 0
            fl = {}
            for ti in range(nt):
                frontier = tl[ti + 1][1] if ti + 1 < nt else 512
                if ti == nt - 1:
                    frontier = 512
                if frontier - lo >= 256 or (ti == nt - 1 and frontier > lo):
                    fl[ti] = (lo, frontier)
                    lo = frontier
            for ti, (p, qs, cm, moff) in enumerate(tl):
                flat.append((qg, p, qs, cm, moff, ti == 0, ti == nt - 1,
                             fl.get(ti)))

        with tc.tile_pool(name="sp", bufs=4, space="PSUM") as sp_pool, \
             tc.tile_pool(name="op", bufs=1, space="PSUM") as op_pool, \
             tc.tile_pool(name="dp", bufs=1, space="PSUM") as dp_pool, \
             tc.tile_pool(name="pt", bufs=4) as pt_pool:

            otp = {}
            dnp = {}
            sps = {}
            pts = {}
            started = set()   # PSUM banks that already got their start=True

            def smm(i):
                qg, p, qs, cm, moff, first, last, fl = flat[i]
                sp = sp_pool.tile([128, 512], F32, tag="sp")
                n = 512 - qs
                for dh2 in range(2):
                    nc.tensor.matmul(
                        sp[:, :n],
                        kt_sb[:, dh2, p * 128:(p + 1) * 128],
                        qt_sb[:, dh2, qg * 512 + qs:(qg + 1) * 512],
                        start=(dh2 == 0), stop=(dh2 == 1),
                    )
                sps[i] = sp

            def mask_exp(i):
                qg, p, qs, cm, moff, first, last, fl = flat[i]
                sp = sps[i]
                n = 512 - qs
                m = cm - qs
                pt = pt_pool.tile([128, 512], BF16, tag="pt")
                if cm < 512:   # unmasked part first — no DVE dependency
                    nc.scalar.activation(pt[:, m:n], sp[:, m:n], AF.Exp,
                                         scale=SCALE, bias=pb_sb[:, p:p + 1])
                if m > 0:
                    nc.vector.tensor_tensor(
                        sp[:, :m], sp[:, :m], mask_sb[:, moff:moff + m],
                        op=ALU.add,
                    )
                    nc.scalar.activation(pt[:, :m], sp[:, :m], AF.Exp,
                                         scale=SCALE, bias=pb_sb[:, p:p + 1])
                pts[i] = pt

            def avdn(i):
                qg, p, qs, cm, moff, first, last, fl = flat[i]
                pt = pts.pop(i)
                n = 512 - qs
                m = cm - qs
                ranges = []
                if cm < 512:
                    ranges.append((m, n, cm))
                if m > 0:
                    ranges.append((0, m, qs))
                for r0, r1, q0 in ranges:
                    for dh2 in range(2):
                        st = (qg, "o", dh2) not in started
                        if st:
                            started.add((qg, "o", dh2))
                        nc.tensor.matmul(
                            otp[qg][dh2][:, q0:q0 + (r1 - r0)],
                            v_sb[:, p, dh2 * 128:(dh2 + 1) * 128],
                            pt[:, r0:r1],
                            start=st, stop=last,
                        )
                    st = (qg, "d") not in started
                    if st:
                        started.add((qg, "d"))
                    nc.tensor.matmul(
                        dnp[qg][:, q0:q0 + (r1 - r0)], ones_sb[:],
                        pt[:, r0:r1], start=st, stop=last,
                    )
                sps.pop(i)

            def flush(qg, lo, hi):
                q0 = qg * 512
                nc.scalar.activation(o_sb[:, 0, q0 + lo:q0 + hi],
                                     otp[qg][0][:, lo:hi], AF.Identity)
                nc.vector.tensor_copy(o_sb[:, 1, q0 + lo:q0 + hi],
                                      otp[qg][1][:, lo:hi])
                nc.vector.tensor_copy(dn_sb[0:1, q0 + lo:q0 + hi],
                                      dnp[qg][0:1, lo:hi])
                for dh2 in range(2):
                    nc.sync.dma_start(
                        t["out"][:, dh2 * SQ + q0 + lo:dh2 * SQ + q0 + hi],
                        o_sb[:, dh2, q0 + lo:q0 + hi],
                    )
                nc.scalar.dma_start(t["odn"][:, q0 + lo:q0 + hi],
                                    dn_sb[0:1, q0 + lo:q0 + hi])

            for i, (qg, p, qs, cm, moff, first, last, fl) in enumerate(flat):
                if first:
                    otp[qg] = [
                        op_pool.tile([128, 512], F32, tag=f"ot{d}",
                                     name=f"otp{qg}_{d}")
                        for d in range(2)
                    ]
                    dnp[qg] = dp_pool.tile([128, 512], F32, tag="dn",
                                           name=f"dnp{qg}")
                if i == 0:
                    for j in range(min(LOOK, len(flat))):
                        smm(j)
                        mask_exp(j)
                if i + LOOK < len(flat):
                    smm(i + LOOK)
                    mask_exp(i + LOOK)
                avdn(i)
                if fl is not None:
                    flush(qg, fl[0], fl[1])


def _get_program(plan):
    k = plan["key"]
    if k not in _prog_cache:
        _prog_cache[k] = _build_program(plan)
    return _prog_cache[k]


def kernel(x, causal_mask, padding_mask, W_Q, b_Q, W_K, b_K, W_V, b_V):
    global LAST_RESULTS
    from concourse.bass_utils import run_bass_kernel_spmd

    import ml_dtypes

    Bb = ml_dtypes.bfloat16
    x = np.ascontiguousarray(x, dtype=np.float32)
    causal = np.asarray(causal_mask) != 0            # [S, S] attend where True
    pad = np.asarray(padding_mask)                   # [B, S]  True = masked key

    plan = _build_plan(causal, pad)
    kpos, nv, VK, NKT = plan["kpos"], plan["nv"], plan["VK"], plan["NKT"]
    groups, visits, totm = plan["groups"], plan["visits"], plan["totm"]

    def tile_w(W):
        W = np.asarray(W, dtype=np.float32)
        return np.ascontiguousarray(
            W.reshape(EC, 128, DH).transpose(1, 0, 2).reshape(128, EC * DH)
        ).astype(Bb)

    wq_t, wk_t, wv_t = tile_w(W_Q), tile_w(W_K), tile_w(W_V)
    bq = np.ascontiguousarray(np.asarray(b_Q, np.float32).reshape(2, 128).T)
    bk = np.ascontiguousarray(np.asarray(b_K, np.float32).reshape(2, 128).T)
    onesm = np.ones((128, 128), dtype=Bb)

    in_maps = []
    for c in range(8):
        b, h = c // 2, c % 2
        qp = _qpos(h)
        vT = plan["vTs"][c]

        # xq: [p][g][c][s] flat, bf16
        xq = (x[b][qp].reshape(2, 512, EC, 128)
              .transpose(3, 0, 2, 1).reshape(128, EC * SQ))
        xq = np.ascontiguousarray(xq).astype(Bb)

        # xkv: compacted keys, ragged groups, [p][(g,c,s)] flat
        xs = np.zeros((VK, E), dtype=np.float32)
        xs[: nv[b]] = x[b][kpos[b]]
        parts = []
        for goff, w in groups:
            parts.append(xs[goff:goff + w].reshape(w, EC, 128)
                         .transpose(2, 1, 0).reshape(128, EC * w))
        xkv = np.ascontiguousarray(np.concatenate(parts, axis=1)).astype(Bb)

        # per-partition exp bias: kill dead key rows (slot >= nv)
        pbias = np.zeros((128, NKT), dtype=np.float32)
        for p in range(NKT):
            dead = np.arange(p * 128, (p + 1) * 128) >= nv[b]
            pbias[dead, p] = NEG
        # 2D masks: causal-partial band [qs, cm) of each visited tile
        mk = np.zeros((128, totm), dtype=np.float32)
        for qg, tl in visits:
            for p, qs, cm, moff in tl:
                if cm > qs:
                    vb = vT[p * 128:(p + 1) * 128,
                            qg * 512 + qs:qg * 512 + cm]
                    mk[:, moff:moff + cm - qs] = np.where(vb, 0.0,
                                                          np.float32(NEG))
        mk = mk.astype(Bb)

        in_maps.append({
            "xq": xq, "xkv": xkv,
            "wq": wq_t, "wk": wk_t, "wv": wv_t,
            "bq": bq, "bk": bk, "pbias": pbias,
            "ones": onesm, "maskc": mk,
        })

    nc = _get_program(plan)
    res = run_bass_kernel_spmd(nc, in_maps, list(range(8)), trace=TRACE)
    LAST_RESULTS = res

    bv = np.asarray(b_V, dtype=np.float32)
    outp = np.empty((B, S, DH), dtype=np.float32)
    for c in range(8):
        b, h = c // 2, c % 2
        qp = _qpos(h)
        num = res.results[c]["out"].astype(np.float32).reshape(128, 2, SQ)
        dn = res.results[c]["odn"][0]                      # [q]
        o = num.transpose(2, 1, 0).reshape(SQ, DH) / dn[:, None] + bv
        outp[b][qp] = o
    return outp


# revision 22
# speedup vs baseline: 1.0889x; 1.0889x over previous
"""Trainium2 Bass kernel for nn_AttentionHead (B=4, S=2048, E=2048, DH=256).

v3: padded-key compaction + bf16 + host-side normalization + tuned overlap.

Sharding: 8 cores = (batch b, query-parity h). Core (b, h) owns the 1024
queries in 128-row blocks {h, h+2, ..., h+14} and attends over the COMPACTED
key list of batch b (padding-masked keys dropped entirely — exact: they get
zero softmax weight). ~half the keys are padding, so scores/AV/denominator
work halves.

All heavy matmuls run in bf16. Host packs x/weights so every DMA is
8KB-contiguous per partition; input streams are spread over the three DMA
queues (SP: xkv, ACT: xq, Pool/SWDGE: weights+masks) in consumption order.

Scores are computed transposed (S^T[k, q]). Padding-tail key rows are masked
via the per-partition exp bias (free on ACT); only the causal-partial column
band of each tile gets a 2D mask add on DVE. exp/AV are split at the band
edge so the unmasked part never waits on DVE. Phase A is software-pipelined
(score matmuls run 2 tiles ahead of the P*V consumers). The device returns
the UNNORMALIZED numerator O^T = V^T P and denominators; the host divides
and adds b_V.
"""
import sys

sys.path.insert(0, "/opt/trn_rl_repo")

import numpy as np

import concourse.bacc as bacc
import concourse.mybir as mybir
import concourse.tile as tile

F32 = mybir.dt.float32
BF16 = mybir.dt.bfloat16
AF = mybir.ActivationFunctionType
ALU = mybir.AluOpType

B, S, E, DH = 4, 2048, 2048, 256
SQ = S // 2          # queries per core
EC = E // 128        # contraction chunks (16)
NEG = -1.0e30
SCALE = 1.0 / 16.0   # 1/sqrt(DH)

TRACE = False
LAST_RESULTS = None

_prog_cache = {}


def _qpos(h):
    """Global query positions owned by core-parity h (interleaved blocks)."""
    return np.concatenate(
        [np.arange((2 * j + h) * 128, (2 * j + h + 1) * 128) for j in range(8)]
    )


def _build_plan(causal, pad):
    """Static SPMD structure (shared by all cores) + per-core mask data."""
    kpos = [np.where(~pad[b])[0] for b in range(B)]
    nv = [len(k) for k in kpos]
    NKT = (max(nv) + 127) // 128
    VK = NKT * 128

    groups = []
    off = 0
    while off < VK:
        w = min(512, VK - off)
        groups.append((off, w))
        off += w

    # per-core transposed validity [key-slot, query-col]
    vTs = []
    for c in range(8):
        b, h = c // 2, c % 2
        qp = _qpos(h)
        vT = np.zeros((VK, SQ), dtype=bool)
        vT[: nv[b]] = causal[np.ix_(qp, kpos[b])].T
        vTs.append(vT)

    # visit set: union over cores.
    #   qs = first query col with any valid entry (min over cores)
    #   cm = first query col from which every core's tile-cols are fully valid
    #        among live rows (rows < nv; dead rows are handled by the exp
    #        bias) — only [qs, cm) needs the 2D mask add.
    visits = []   # [(qg, [(p, qs, cm, moff), ...]), ...]
    moff = 0
    for qg in range(2):
        tl = []
        for p in range(NKT):
            anyv = np.zeros(512, dtype=bool)
            fullv = np.ones(512, dtype=bool)
            for ci, vT in enumerate(vTs):
                b = ci // 2
                live = min(128, max(0, nv[b] - p * 128))
                seg = vT[p * 128:(p + 1) * 128, qg * 512:(qg + 1) * 512]
                anyv |= seg.any(axis=0)
                fullv &= seg[:live].all(axis=0)
            if not anyv.any():
                continue
            qs = int(np.argmax(anyv))
            # cm: first col such that fullv[cm:] is all True
            idx = np.where(~fullv)[0]
            cm = int(idx[-1]) + 1 if len(idx) else qs
            cm = max(cm, qs)
            tl.append((p, qs, cm, moff))
            moff += cm - qs
        visits.append((qg, tl))
    totm = max(moff, 8)

    key = (VK, tuple(groups),
           tuple((qg, tuple(tl)) for qg, tl in visits))
    return dict(kpos=kpos, nv=nv, NKT=NKT, VK=VK, groups=groups,
                visits=visits, totm=totm, vTs=vTs, key=key)


def _build_program(plan):
    nc = bacc.Bacc("TRN2", target_bir_lowering=False, debug=False, num_devices=8)
    VK, NKT, totm = plan["VK"], plan["NKT"], plan["totm"]

    t = {}
    t["xq"] = nc.dram_tensor("xq", [128, EC * SQ], BF16, kind="ExternalInput").ap()
    t["xkv"] = nc.dram_tensor("xkv", [128, EC * VK], BF16, kind="ExternalInput").ap()
    t["wq"] = nc.dram_tensor("wq", [128, EC * DH], BF16, kind="ExternalInput").ap()
    t["wk"] = nc.dram_tensor("wk", [128, EC * DH], BF16, kind="ExternalInput").ap()
    t["wv"] = nc.dram_tensor("wv", [128, EC * DH], BF16, kind="ExternalInput").ap()
    t["bq"] = nc.dram_tensor("bq", [128, 2], F32, kind="ExternalInput").ap()
    t["bk"] = nc.dram_tensor("bk", [128, 2], F32, kind="ExternalInput").ap()
    t["pbias"] = nc.dram_tensor("pbias", [128, NKT], F32, kind="ExternalInput").ap()
    t["ones"] = nc.dram_tensor("ones", [128, 128], BF16, kind="ExternalInput").ap()
    t["maskc"] = nc.dram_tensor("maskc", [128, totm], BF16, kind="ExternalInput").ap()
    t["out"] = nc.dram_tensor("out", [128, 2 * SQ], BF16, kind="ExternalOutput").ap()
    t["odn"] = nc.dram_tensor("odn", [1, SQ], F32, kind="ExternalOutput").ap()

    with tile.TileContext(nc) as tc:
        _emit(nc, tc, t, plan)
    nc.compile()
    return nc


def _emit(nc, tc, t, plan):
    from contextlib import ExitStack

    VK, NKT = plan["VK"], plan["NKT"]
    groups, visits, totm = plan["groups"], plan["visits"], plan["totm"]

    with ExitStack() as ctx:
        const = ctx.enter_context(tc.tile_pool(name="const", bufs=1))
        persist = ctx.enter_context(tc.tile_pool(name="persist", bufs=1))

        # ---- PE warm-up: ~4us of dummy matmuls while the first input
        # chunks stream in, so the HAM un-throttles before real work.
        warm_a = const.tile([128, 128], BF16, tag="warm_a")
        warm_b = const.tile([128, 512], BF16, tag="warm_b")
        nc.vector.memset(warm_a[:], 0.0)
        nc.vector.memset(warm_b[:], 0.0)
        with tc.tile_pool(name="warmp", bufs=1, space="PSUM") as warmp:
            wp = warmp.tile([128, 512], F32, tag="wp")
            for _ in range(10):
                nc.tensor.matmul(wp[:], warm_a[:], warm_b[:],
                                 start=True, stop=True)

        # ---- constants --------------------------------------------------
        # deadline-ordered across the three DMA queues:
        #   ACT ring:  wk head, wk rest, then xq (emitted in load_xq)
        #   Pool ring: wq, wv, small consts
        #   SP ring:   xkv chunks, then masks
        wk_sb = const.tile([128, EC, DH], BF16, tag="wk")
        wq_sb = const.tile([128, EC, DH], BF16, tag="wq")
        wv_sb = const.tile([128, EC, DH], BF16, tag="wv")
        wk_r = t["wk"].rearrange("p (c d) -> p c d", c=EC)
        nc.scalar.dma_start(wk_sb[:, 0:2, :], wk_r[:, 0:2, :])
        nc.scalar.dma_start(wk_sb[:, 2:8, :], wk_r[:, 2:8, :])
        nc.scalar.dma_start(wk_sb[:, 8:, :], wk_r[:, 8:, :])
        nc.gpsimd.dma_start(wq_sb[:], t["wq"].rearrange("p (c d) -> p c d", c=EC))
        nc.gpsimd.dma_start(wv_sb[:], t["wv"].rearrange("p (c d) -> p c d", c=EC))
        bq_sb = const.tile([128, 2], F32, tag="bq")
        bk_sb = const.tile([128, 2], F32, tag="bk")
        pb_sb = const.tile([128, NKT], F32, tag="pb")
        ones_sb = const.tile([128, 128], BF16, tag="ones")
        nc.gpsimd.dma_start(bq_sb[:], t["bq"][:])
        nc.gpsimd.dma_start(bk_sb[:], t["bk"][:])
        nc.gpsimd.dma_start(pb_sb[:], t["pbias"][:])
        nc.gpsimd.dma_start(ones_sb[:], t["ones"][:])
        mask_sb = const.tile([128, totm], BF16, tag="mask")

        # ---- persistent SBUF tensors
        kt_sb = persist.tile([128, 2, VK], BF16, tag="ktp")
        qt_sb = persist.tile([128, 2, SQ], BF16, tag="qtp")
        v_sb = persist.tile([128, NKT, DH], BF16, tag="vp")
        o_sb = persist.tile([128, 2, SQ], BF16, tag="osb")
        dn_sb = persist.tile([128, SQ], F32, tag="dnsb")

        xq_r = t["xq"].rearrange("p (g c s) -> p g c s", g=2, c=EC)

        # ---- phase P: projections -------------------------------------
        # Per super-group: K (2 dh-halves), Q (2 dh-halves), V (4 key
        # tiles).  xkv chunks stream on SP, xq on ACT, weights on Pool.
        with tc.tile_pool(name="xkv", bufs=2) as xkv_pool, \
             tc.tile_pool(name="xq", bufs=2) as xq_pool, \
             tc.tile_pool(name="pp", bufs=2, space="PSUM") as pp, \
             tc.tile_pool(name="vp", bufs=2, space="PSUM") as vp:

            def load_xkv(gi, csplit):
                goff, w = groups[gi]
                xkt = xkv_pool.tile([128, EC, 512], BF16, tag="xkt")
                c0 = 0
                for cw in csplit:
                    foff = goff * EC + c0 * w
                    nc.sync.dma_start(
                        xkt[:, c0:c0 + cw, :w],
                        t["xkv"][:, foff:foff + cw * w].rearrange(
                            "p (c s) -> p c s", c=cw),
                    )
                    c0 += cw
                return xkt

            def load_xq(g):
                xqt = xq_pool.tile([128, EC, 512], BF16, tag="xqt")
                for ch in range(2):
                    nc.scalar.dma_start(
                        xqt[:, ch * 8:(ch + 1) * 8, :],
                        xq_r[:, g, ch * 8:(ch + 1) * 8, :],
                    )
                return xqt

            def proj(w_sb, b_sb, dst, dh2, xt, w):
                ps = pp.tile([128, 512], F32, tag="pp")
                for e in range(EC):
                    nc.tensor.matmul(
                        ps[:, :w], w_sb[:, e, dh2 * 128:(dh2 + 1) * 128],
                        xt[:, e, :w], start=(e == 0), stop=(e == EC - 1),
                    )
                nc.scalar.activation(dst, ps[:, :w], AF.Identity,
                                     bias=b_sb[:, dh2:dh2 + 1])

            def vproj(xt, goff, w):
                for kt4 in range(w // 128):
                    psv = vp.tile([128, DH], F32, tag="vps")
                    for e in range(EC):
                        nc.tensor.matmul(
                            psv[:], xt[:, e, kt4 * 128:(kt4 + 1) * 128],
                            wv_sb[:, e, :], start=(e == 0), stop=(e == EC - 1),
                        )
                    nc.vector.tensor_copy(v_sb[:, goff // 128 + kt4, :], psv[:])

            # prefetch order on each ring == consumption order
            xk0 = load_xkv(0, [2, 2, 4, 4, 4])
            xq0 = load_xq(0)
            xk1 = load_xkv(1, [8, 8]) if len(groups) > 1 else None
            xq1 = load_xq(1)
            xk2 = load_xkv(2, [16]) if len(groups) > 2 else None
            # masks ride the SP ring after all xkv (needed only in phase A)
            nc.sync.dma_start(mask_sb[:], t["maskc"][:])

            for gi, xkt in enumerate([xk0, xk1, xk2][:len(groups)]):
                goff, w = groups[gi]
                for dh2 in range(2):
                    proj(wk_sb, bk_sb, kt_sb[:, dh2, goff:goff + w], dh2, xkt, w)
                if gi < 2:
                    xqt = xq0 if gi == 0 else xq1
                    for dh2 in range(2):
                        proj(wq_sb, bq_sb, qt_sb[:, dh2, gi * 512:(gi + 1) * 512],
                             dh2, xqt, 512)
                vproj(xkt, goff, w)

        # ---- phase A: attention ---------------------------------------
        # Flatten tiles across both query-groups; software-pipeline: score
        # matmuls run LOOKAHEAD tiles ahead of the mask/exp/AV consumers.
        LOOK = 2
        flat = []
        for qg, tl in visits:
            nt = len(tl)
            # staged flush: cols < next tile's qs are final after this tile;
            # flush in >=256-col pieces so the tail only handles the last one
            lo = 0
            fl = {}
            for ti in range(nt):
                frontier = tl[ti + 1][1] if ti + 1 < nt else 512
                if ti == nt - 1:
                    frontier = 512
                if frontier - lo >= 256 or (ti == nt - 1 and frontier > lo):
                    fl[ti] = (lo, frontier)
                    lo = frontier
            for ti, (p, qs, cm, moff) in enumerate(tl):
                flat.append((qg, p, qs, cm, moff, ti == 0, ti == nt - 1,
                             fl.get(ti)))

        with tc.tile_pool(name="sp", bufs=4, space="PSUM") as sp_pool, \
             tc.tile_pool(name="op", bufs=1, space="PSUM") as op_pool, \
             tc.tile_pool(name="dp", bufs=1, space="PSUM") as dp_pool, \
             tc.tile_pool(name="pt", bufs=4) as pt_pool:

            otp = {}
            dnp = {}
            sps = {}
            pts = {}
            started = set()   # PSUM banks that already got their start=True

            def smm(i):
                qg, p, qs, cm, moff, first, last, fl = flat[i]
                sp = sp_pool.tile([128, 512], F32, tag="sp")
                n = 512 - qs
                for dh2 in range(2):
                    nc.tensor.matmul(
                        sp[:, :n],
                        kt_sb[:, dh2, p * 128:(p + 1) * 128],
                        qt_sb[:, dh2, qg * 512 + qs:(qg + 1) * 512],
                        start=(dh2 == 0), stop=(dh2 == 1),
                    )
                sps[i] = sp

            def mask_exp(i):
                qg, p, qs, cm, moff, first, last, fl = flat[i]
                sp = sps[i]
                n = 512 - qs
                m = cm - qs
                pt = pt_pool.tile([128, 512], BF16, tag="pt")
                if cm < 512:   # unmasked part first — no DVE dependency
                    nc.scalar.activation(pt[:, m:n], sp[:, m:n], AF.Exp,
                                         scale=SCALE, bias=pb_sb[:, p:p + 1])
                if m > 0:
                    nc.vector.tensor_tensor(
                        sp[:, :m], sp[:, :m], mask_sb[:, moff:moff + m],
                        op=ALU.add,
                    )
                    nc.scalar.activation(pt[:, :m], sp[:, :m], AF.Exp,
                                         scale=SCALE, bias=pb_sb[:, p:p + 1])
                pts[i] = pt

            def avdn(i):
                qg, p, qs, cm, moff, first, last, fl = flat[i]
                pt = pts.pop(i)
                n = 512 - qs
                m = cm - qs
                ranges = []
                if cm < 512:
                    ranges.append((m, n, cm))
                if m > 0:
                    ranges.append((0, m, qs))
                for r0, r1, q0 in ranges:
                    for dh2 in range(2):
                        st = (qg, "o", dh2) not in started
                        if st:
                            started.add((qg, "o", dh2))
                        nc.tensor.matmul(
                            otp[qg][dh2][:, q0:q0 + (r1 - r0)],
                            v_sb[:, p, dh2 * 128:(dh2 + 1) * 128],
                            pt[:, r0:r1],
                            start=st, stop=last,
                        )
                    st = (qg, "d") not in started
                    if st:
                        started.add((qg, "d"))
                    nc.tensor.matmul(
                        dnp[qg][:, q0:q0 + (r1 - r0)], ones_sb[:],
                        pt[:, r0:r1], start=st, stop=last,
                    )
                sps.pop(i)

            def flush(qg, lo, hi):
                q0 = qg * 512
                nc.scalar.activation(o_sb[:, 0, q0 + lo:q0 + hi],
                                     otp[qg][0][:, lo:hi], AF.Identity)
                nc.vector.tensor_copy(o_sb[:, 1, q0 + lo:q0 + hi],
                                      otp[qg][1][:, lo:hi])
                nc.vector.tensor_copy(dn_sb[0:1, q0 + lo:q0 + hi],
                                      dnp[qg][0:1, lo:hi])
                for dh2 in range(2):
                    nc.sync.dma_start(
                        t["out"][:, dh2 * SQ + q0 + lo:dh2 * SQ + q0 + hi],
                        o_sb[:, dh2, q0 + lo:q0 + hi],
                    )
                nc.scalar.dma_start(t["odn"][:, q0 + lo:q0 + hi],
                                    dn_sb[0:1, q0 + lo:q0 + hi])

            for i, (qg, p, qs, cm, moff, first, last, fl) in enumerate(flat):
                if first:
                    otp[qg] = [
                        op_pool.tile([128, 512], F32, tag=f"ot{d}",
                                     name=f"otp{qg}_{d}")
                        for d in range(2)
                    ]
                    dnp[qg] = dp_pool.tile([128, 512], F32, tag="dn",
                                           name=f"dnp{qg}")
                if i == 0:
                    for j in range(min(LOOK, len(flat))):
                        smm(j)
                        mask_exp(j)
                if i + LOOK < len(flat):
                    smm(i + LOOK)
                    mask_exp(i + LOOK)
                avdn(i)
                if fl is not None:
                    flush(qg, fl[0], fl[1])


def _get_program(plan):
    k = plan["key"]
    if k not in _prog_cache:
        _prog_cache[k] = _build_program(plan)
    return _prog_cache[k]


def kernel(x, causal_mask, padding_mask, W_Q, b_Q, W_K, b_K, W_V, b_V):
    global LAST_RESULTS
    from concourse.bass_utils import run_bass_kernel_spmd

    import ml_dtypes

    Bb = ml_dtypes.bfloat16
    x = np.ascontiguousarray(x, dtype=np.float32)
    causal = np.asarray(causal_mask) != 0            # [S, S] attend where True
    pad = np.asarray(padding_mask)                   # [B, S]  True = masked key

    plan = _build_plan(causal, pad)
    kpos, nv, VK, NKT = plan["kpos"], plan["nv"], plan["VK"], plan["NKT"]
    groups, visits, totm = plan["groups"], plan["visits"], plan["totm"]

    def tile_w(W):
        W = np.asarray(W, dtype=np.float32)
        return np.ascontiguousarray(
            W.reshape(EC, 128, DH).transpose(1, 0, 2).reshape(128, EC * DH)
        ).astype(Bb)

    wq_t, wk_t, wv_t = tile_w(W_Q), tile_w(W_K), tile_w(W_V)
    bq = np.ascontiguousarray(np.asarray(b_Q, np.float32).reshape(2, 128).T)
    bk = np.ascontiguousarray(np.asarray(b_K, np.float32).reshape(2, 128).T)
    onesm = np.ones((128, 128), dtype=Bb)

    in_maps = []
    for c in range(8):
        b, h = c // 2, c % 2
        qp = _qpos(h)
        vT = plan["vTs"][c]

        # xq: [p][g][c][s] flat, bf16
        xq = (x[b][qp].reshape(2, 512, EC, 128)
              .transpose(3, 0, 2, 1).reshape(128, EC * SQ))
        xq = np.ascontiguousarray(xq).astype(Bb)

        # xkv: compacted keys, ragged groups, [p][(g,c,s)] flat
        xs = np.zeros((VK, E), dtype=np.float32)
        xs[: nv[b]] = x[b][kpos[b]]
        parts = []
        for goff, w in groups:
            parts.append(xs[goff:goff + w].reshape(w, EC, 128)
                         .transpose(2, 1, 0).reshape(128, EC * w))
        xkv = np.ascontiguousarray(np.concatenate(parts, axis=1)).astype(Bb)

        # per-partition exp bias: kill dead key rows (slot >= nv)
        pbias = np.zeros((128, NKT), dtype=np.float32)
        for p in range(NKT):
            dead = np.arange(p * 128, (p + 1) * 128) >= nv[b]
            pbias[dead, p] = NEG
        # 2D masks: causal-partial band [qs, cm) of each visited tile
        mk = np.zeros((128, totm), dtype=np.float32)
        for qg, tl in visits:
            for p, qs, cm, moff in tl:
                if cm > qs:
                    vb = vT[p * 128:(p + 1) * 128,
                            qg * 512 + qs:qg * 512 + cm]
                    mk[:, moff:moff + cm - qs] = np.where(vb, 0.0,
                                                          np.float32(NEG))
        mk = mk.astype(Bb)

        in_maps.append({
            "xq": xq, "xkv": xkv,
            "wq": wq_t, "wk": wk_t, "wv": wv_t,
            "bq": bq, "bk": bk, "pbias": pbias,
            "ones": onesm, "maskc": mk,
        })

    nc = _get_program(plan)
    res = run_bass_kernel_spmd(nc, in_maps, list(range(8)), trace=TRACE)
    LAST_RESULTS = res

    bv = np.asarray(b_V, dtype=np.float32)
    outp = np.empty((B, S, DH), dtype=np.float32)
    for c in range(8):
        b, h = c // 2, c % 2
        qp = _qpos(h)
        num = res.results[c]["out"].astype(np.float32).reshape(128, 2, SQ)
        dn = res.results[c]["odn"][0]                      # [q]
        o = num.transpose(2, 1, 0).reshape(SQ, DH) / dn[:, None] + bv
        outp[b][qp] = o
    return outp
